# revision 4
# baseline (speedup 1.0000x reference)
"""GGNN message passing + bilinear readout on 8 TRN2 NeuronCores.

Problem: nn_BaselineModel_36687610642509 (gnn_message_passing).

reference:
    for 8 iters:  per_edge = einsum('sd,edh->seh', h, W_msg)
                  messages = einsum('ste,seh->th', edge, per_edge) + b_msg
                  h = GRU(h, messages)          (Wi, Wh, b_gru)
    logits = einsum('id,de,je->ij', h, A_readout, h)

Distribution (1D node parallelism, 8 cores, DESTINATION-sharded):
    Reassociate the message computation as
        messages[t,:] = sum_e (edge_e^T @ h) @ W_e
    so the expensive contraction runs directly between the edge tensor and
    the raw node embeddings h -- no per-edge-type transform of all senders
    is ever materialized.  Core k owns destination nodes t_k = [256k, 256k+256):
    - edge[:, t_k, :] lives in SBUF for the whole kernel (bf16, 8 MiB),
      laid out as 16 sender-block tiles [128, E*256].
    - h is replicated in node-major layout [2048, 128]; each core computes
      u = edge_k^T h for its own destinations (PSUM-accumulated over the 16
      sender blocks), then messages^T = sum_e W_e^T u_e (8 tiny matmuls).
    - GRU update runs shard-locally in transposed [D, 256] layout.
    - One 64 KiB AllGather of the updated h shard per iteration (floor-bound,
      ~5us) replaces the baseline's 512 KiB ReduceScatter (~14us); h_new is
      transposed to node-major with 2 tensor-engine transposes before the AG.
    - The readout needs h^T replicated, so the last exchange gathers the
      transposed shard instead; each core emits its 256 rows of the logits.
All matmul operands are bf16 (fp32 PSUM accumulation).
"""

import sys

for _p in ("/opt/trn_rl_repo",):
    if _p not in sys.path:
        sys.path.insert(0, _p)

import numpy as np
import ml_dtypes

import concourse.bacc as bacc
import concourse.tile as tile
import concourse.mybir as mybir
from concourse import bass_utils

dt = mybir.dt
AF = mybir.ActivationFunctionType

N_CORES = 8
N = 2048          # nodes
D = 128           # embedding dim
E = 8             # edge channels
ITERS = 8
S = N // N_CORES  # 256 nodes per core
NB = N // D       # 16 sender blocks
RG = [list(range(N_CORES))]


def build_nc(reps=1, skip_coll=False, **_legacy):
    nc = bacc.Bacc("TRN2", target_bir_lowering=False, debug=False,
                   num_devices=N_CORES)

    edgeu = nc.dram_tensor("edgeu", [N, E * S], dt.bfloat16, kind="ExternalInput")
    h0f = nc.dram_tensor("h0f", [N, D], dt.bfloat16, kind="ExternalInput")
    h0t = nc.dram_tensor("h0t", [D, S], dt.bfloat16, kind="ExternalInput")
    wmsg = nc.dram_tensor("wmsg", [D, E * D], dt.bfloat16, kind="ExternalInput")
    wi = nc.dram_tensor("wi", [D, 3 * D], dt.bfloat16, kind="ExternalInput")
    wh = nc.dram_tensor("wh", [D, 3 * D], dt.bfloat16, kind="ExternalInput")
    bias = nc.dram_tensor("bias", [D, 3], dt.float32, kind="ExternalInput")
    aro = nc.dram_tensor("aro", [D, D], dt.bfloat16, kind="ExternalInput")
    ident = nc.dram_tensor("ident", [D, D], dt.bfloat16, kind="ExternalInput")
    out = nc.dram_tensor("out", [S, N], dt.float32, kind="ExternalOutput")

    with tile.TileContext(nc) as tc:
        with (
            tc.tile_pool(name="const", bufs=2) as cpool,
            tc.tile_pool(name="sb", bufs=2) as spool,
            tc.tile_pool(name="stage", bufs=2) as stpool,
            tc.tile_pool(name="u_ps", bufs=2, space="PSUM") as u_ps,
            tc.tile_pool(name="mm_ps", bufs=2, space="PSUM") as mm_ps,
            tc.tile_pool(name="gru_ps", bufs=2, space="PSUM") as gru_ps,
            tc.tile_pool(name="dram", bufs=2, space="DRAM") as dram,
        ):
            for rep in range(reps):
                # ---- constants (edge shard resident in SBUF all kernel;
                #      bufs=2 so the next rep's load overlaps this rep) ----
                edge_sb = []
                for b in range(NB):
                    t = cpool.tile([D, E * S], dt.bfloat16, tag=f"edge{b}")
                    nc.sync.dma_start(t[:], edgeu.ap()[b * D:(b + 1) * D, :])
                    edge_sb.append(t)
                wmsg_sb = cpool.tile([D, E * D], dt.bfloat16, tag="wmsg")
                nc.sync.dma_start(wmsg_sb[:], wmsg.ap())
                wi_sb = cpool.tile([D, 3 * D], dt.bfloat16, tag="wi")
                nc.sync.dma_start(wi_sb[:], wi.ap())
                wh_sb = cpool.tile([D, 3 * D], dt.bfloat16, tag="wh")
                nc.sync.dma_start(wh_sb[:], wh.ap())
                bias_sb = cpool.tile([D, 3], dt.float32, tag="bias")
                nc.sync.dma_start(bias_sb[:], bias.ap())
                aro_sb = cpool.tile([D, D], dt.bfloat16, tag="aro")
                nc.sync.dma_start(aro_sb[:], aro.ap())
                ident_sb = cpool.tile([D, D], dt.bfloat16, tag="ident")
                nc.sync.dma_start(ident_sb[:], ident.ap())

                # ---- initial state ----
                hf = []   # full h, node-major: tile j = shard j, [t', (i2,d)]
                for j in range(N_CORES):
                    t = spool.tile([D, 2 * D], dt.bfloat16, tag=f"hf{j}")
                    for i2 in range(2):
                        r0 = j * S + i2 * D
                        nc.sync.dma_start(t[:, i2 * D:(i2 + 1) * D],
                                          h0f.ap()[r0:r0 + D, :])
                    hf.append(t)
                hT = spool.tile([D, S], dt.bfloat16, tag="hT")
                nc.sync.dma_start(hT[:], h0t.ap())

                for it in range(ITERS):
                    # ---- u[d, (e,t')] = sum_s h[s,d] edge[s, (e,t')] ----
                    # two 1024-col halves ping-pong 2 PSUM banks each; the
                    # copy of half 0 overlaps the accumulation of half 1.
                    ubf = spool.tile([D, E * S], dt.bfloat16, tag="ubf")
                    for half in range(2):
                        U = u_ps.tile([D, E * S // 2], dt.float32, tag="U",
                                      bufs=2)
                        for b in range(NB):
                            lhsT = hf[b >> 1][:, (b & 1) * D:((b & 1) + 1) * D]
                            for c in range(2):
                                o = half * 1024 + c * 512
                                nc.tensor.matmul(
                                    U[:, c * 512:(c + 1) * 512],
                                    lhsT,
                                    edge_sb[b][:, o:o + 512],
                                    start=(b == 0), stop=(b == NB - 1),
                                )
                        for c in range(2):
                            o = half * 1024 + c * 512
                            nc.vector.tensor_copy(ubf[:, o:o + 512],
                                                  U[:, c * 512:(c + 1) * 512])

                    # ---- messages^T = sum_e W_e^T u_e  [D, S] ----
                    M = mm_ps.tile([D, S], dt.float32, tag="M",
                                     bufs=1)
                    for e in range(E):
                        nc.tensor.matmul(M[:],
                                         wmsg_sb[:, e * D:(e + 1) * D],
                                         ubf[:, e * S:(e + 1) * S],
                                         start=(e == 0), stop=(e == E - 1))
                    msgs_bf = spool.tile([D, S], dt.bfloat16, tag="msgsbf")
                    nc.vector.tensor_copy(msgs_bf[:], M[:])

                    # ---- GRU (transposed layout, biases folded) ----
                    new_hT = spool.tile([D, S], dt.bfloat16, tag="hT")
                    gate = []
                    for g in range(2):
                        gp = gru_ps.tile([D, S], dt.float32, tag="gru")
                        nc.tensor.matmul(gp[:], wi_sb[:, g * D:(g + 1) * D],
                                         msgs_bf[:], start=True, stop=False)
                        nc.tensor.matmul(gp[:], wh_sb[:, g * D:(g + 1) * D],
                                         hT[:], start=False, stop=True)
                        gs = stpool.tile([D, S], dt.float32, tag=f"g{g}")
                        nc.scalar.activation(gs[:], gp[:], AF.Sigmoid,
                                             bias=bias_sb[:, g:g + 1])
                        gate.append(gs)
                    r_g, z_g = gate

                    inp = gru_ps.tile([D, S], dt.float32, tag="gru")
                    nc.tensor.matmul(inp[:], wi_sb[:, 2 * D:3 * D], msgs_bf[:],
                                     start=True, stop=True)
                    hnp = gru_ps.tile([D, S], dt.float32, tag="gru")
                    nc.tensor.matmul(hnp[:], wh_sb[:, 2 * D:3 * D], hT[:],
                                     start=True, stop=True)
                    t1 = stpool.tile([D, S], dt.float32, tag="t1")
                    nc.vector.tensor_mul(t1[:], r_g[:], hnp[:])
                    t2 = stpool.tile([D, S], dt.float32, tag="t2")
                    nc.vector.tensor_add(t2[:], t1[:], inp[:])
                    n_sb = stpool.tile([D, S], dt.float32, tag="n")
                    nc.scalar.activation(n_sb[:], t2[:], AF.Tanh,
                                         bias=bias_sb[:, 2:3])
                    # h_new = n + z * (h - n)
                    d1 = stpool.tile([D, S], dt.float32, tag="d1")
                    nc.vector.tensor_sub(d1[:], hT[:], n_sb[:])
                    d2 = stpool.tile([D, S], dt.float32, tag="d2")
                    nc.vector.tensor_mul(d2[:], z_g[:], d1[:])
                    nc.vector.tensor_add(new_hT[:], n_sb[:], d2[:])
                    hT = new_hT

                    if it < ITERS - 1:
                        # transpose h_new^T -> node-major, AllGather shards
                        hsend = spool.tile([D, 2 * D], dt.bfloat16, tag="hsend")
                        for i2 in range(2):
                            tp = mm_ps.tile([D, D], dt.bfloat16, tag="M",
                                             bufs=1)
                            nc.tensor.transpose(
                                tp[:], hT[:, i2 * D:(i2 + 1) * D], ident_sb[:])
                            nc.vector.tensor_copy(
                                hsend[:, i2 * D:(i2 + 1) * D], tp[:])
                        agin = dram.tile([S, D], dt.bfloat16, tag="agin")
                        for i2 in range(2):
                            nc.sync.dma_start(agin[i2 * D:(i2 + 1) * D, :],
                                              hsend[:, i2 * D:(i2 + 1) * D])
                        agout = dram.tile([N, D], dt.bfloat16, tag="agout")
                        if not skip_coll:
                            nc.gpsimd.collective_compute(
                                "AllGather", mybir.AluOpType.bypass,
                                replica_groups=RG,
                                ins=[agin.opt()], outs=[agout.opt()],
                            )
                        hf = []
                        for j in range(N_CORES):
                            t = spool.tile([D, 2 * D], dt.bfloat16,
                                           tag=f"hf{j}")
                            for i2 in range(2):
                                if skip_coll:
                                    src = agin[i2 * D:(i2 + 1) * D, :]
                                else:
                                    r0 = j * S + i2 * D
                                    src = agout[r0:r0 + D, :]
                                nc.sync.dma_start(
                                    t[:, i2 * D:(i2 + 1) * D], src)
                            hf.append(t)

                # ---- readout: gather h^T, logits rows = (h_k A) @ h^T ----
                agin2 = dram.tile([D, S], dt.bfloat16, tag="agin2")
                nc.sync.dma_start(agin2[:], hT[:])
                agout2 = dram.tile([N_CORES * D, S], dt.bfloat16, tag="agout2")
                if not skip_coll:
                    nc.gpsimd.collective_compute(
                        "AllGather", mybir.AluOpType.bypass,
                        replica_groups=RG,
                        ins=[agin2.opt()], outs=[agout2.opt()],
                    )
                hTf = spool.tile([D, N], dt.bfloat16, tag="hTf")
                for j in range(N_CORES):
                    src = agin2[:] if skip_coll else agout2[j * D:(j + 1) * D, :]
                    nc.sync.dma_start(hTf[:, j * S:(j + 1) * S], src)

                yp = mm_ps.tile([D, S], dt.float32, tag="M", bufs=1)
                nc.tensor.matmul(yp[:], aro_sb[:], hT[:], start=True, stop=True)
                yb = spool.tile([D, S], dt.bfloat16, tag="yb")
                nc.vector.tensor_copy(yb[:], yp[:])

                for i2 in range(2):
                    for jc in range(4):
                        lp = mm_ps.tile([D, 512], dt.float32, tag="L",
                                          bufs=1)
                        nc.tensor.matmul(lp[:],
                                         yb[:, i2 * D:(i2 + 1) * D],
                                         hTf[:, jc * 512:(jc + 1) * 512],
                                         start=True, stop=True)
                        ost = stpool.tile([D, 512], dt.float32, tag="ost")
                        nc.vector.tensor_copy(ost[:], lp[:])
                        nc.sync.dma_start(
                            out.ap()[i2 * D:(i2 + 1) * D,
                                     jc * 512:(jc + 1) * 512],
                            ost[:])

    nc.compile()
    return nc


def make_in_maps(node_embeddings, edge_embeddings, W_msg, b_msg, Wi, Wh,
                 b_gru, A_readout):
    bf16 = ml_dtypes.bfloat16
    wmsg_b = np.ascontiguousarray(
        W_msg.transpose(1, 0, 2).reshape(D, E * D)).astype(bf16)
    wi_b = np.ascontiguousarray(Wi).astype(bf16)
    wh_b = np.ascontiguousarray(Wh).astype(bf16)
    # messages enter the GRU only through  gi = (raw_msgs + b_msg) @ Wi + b_gru,
    # so fold b_msg into a per-gate bias (fp32, exact).
    b_eff = (b_msg.astype(np.float64) @ Wi.astype(np.float64)
             + b_gru.astype(np.float64)).astype(np.float32)
    bias_b = np.ascontiguousarray(b_eff.reshape(3, D).T)  # [D, 3]
    aro_b = np.ascontiguousarray(A_readout).astype(bf16)
    ident_b = np.eye(D, dtype=bf16)
    h0f_b = np.ascontiguousarray(node_embeddings).astype(bf16)

    in_maps = []
    for k in range(N_CORES):
        sl = slice(k * S, (k + 1) * S)
        # edgeu[s, e*S + t'] = edge[s, k*S + t', e]
        ek = np.ascontiguousarray(
            edge_embeddings[:, sl, :].transpose(0, 2, 1).reshape(N, E * S)
        ).astype(bf16)
        h0t_b = np.ascontiguousarray(node_embeddings[sl].T).astype(bf16)
        in_maps.append({
            "edgeu": ek, "h0f": h0f_b, "h0t": h0t_b, "wmsg": wmsg_b,
            "wi": wi_b, "wh": wh_b, "bias": bias_b, "aro": aro_b,
            "ident": ident_b,
        })
    return in_maps


_cache = {}


def kernel(node_embeddings, edge_embeddings, W_msg, b_msg, Wi, Wh, b_gru,
           A_readout):
    if "nc" not in _cache:
        _cache["nc"] = build_nc(reps=1)
    nc = _cache["nc"]
    in_maps = make_in_maps(node_embeddings, edge_embeddings, W_msg, b_msg,
                           Wi, Wh, b_gru, A_readout)
    res = bass_utils.run_bass_kernel_spmd(
        nc, in_maps, core_ids=list(range(N_CORES)))
    return np.concatenate([res.results[k]["out"] for k in range(N_CORES)],
                          axis=0)
